# revision 28
# baseline (speedup 1.0000x reference)
"""Trainium2 Bass kernel for GroupedQueryAttention (anti-causal mask variant).

Reference semantics (B=2, S=2048, D=4096, 32 Q heads, 4 KV heads, dk=128):
  Q = x@Wq, K = x@Wk, V = x@Wv (heads split), GQA repeat KV x8.
  scores = Q K^T / sqrt(dk); mask = triu(ones, k=1); scores = where(mask==0, -1e9, scores)
    -> keeps STRICT UPPER triangle (k > q, anti-causal). Rows with no valid key
       (q == S-1) become a uniform softmax over all S keys.
  out = softmax(scores) @ V; out = out @ Wo.

Sharding: 8 cores, 4 Q heads + their 1 shared KV head per core. Each core
computes a partial out = attn_heads @ Wo_rows_slice; host sums the 8 partials.

Per-core kernel design (bf16 operands, fp32 PSUM accumulation):
  - x is pre-cast to bf16 on the host (inputs stay fp32 at the kernel()
    boundary); x^T tiles via PE transposes (bf16, 1 cycle/row) + DVE/ACT
    copies out of PSUM.
  - Q^T/K^T/V^T projections in [dk, seq] layout (lhsT = bf16 W chunk, FWL).
  - scores computed TRANSPOSED: sT[k, q] = K^T chunk (lhsT) x Q^T (rhs), so
    softmax denominator is a partition-dim sum (ones-matmul) and the AV matmul
    out^T[dk, q] = V chunk (lhsT) x P^T (rhs) accumulates with N=512 and lands
    already transposed for the Wo projection.
  - exp on ACT over CHUNK PAIRS ([128,1024] spanning two PSUM banks), bf16
    out; masking applied POST-exp as cheap bf16 multiplies on the DVE
    (pt *= M01 gives exact zeros, matching exp(-1e9) -> 0). For the LAST q
    block the reference's fully-masked rows need uniform weights, so there
    pt = exp(s)*M01 + exp(-30)*(1-M01), and the skipped blocks' contributions
    are added analytically: r += n_skip*128*exp(-30), out^T += exp(-30)*cumsumV.
"""

import sys
from contextlib import ExitStack

import numpy as np

for _p in ("/opt/trn_rl_repo",):
    if _p not in sys.path:
        sys.path.insert(0, _p)

import bass_rust
import concourse.bass as bass
import concourse.mybir as mybir
import concourse.tile as tile
from concourse.masks import make_identity


def _split_multiwaits(nc):
    """This walrus build encodes at most ONE sem wait per instruction.
    Tile's wait-assignment can attach several; hoist the extras onto fresh
    single-wait NoOps emitted immediately before the instruction on the same
    engine stream. Tile emits instructions in schedule order, so every wait's
    producer precedes the waiting instruction in-stream and the stall cannot
    deadlock."""
    for fn in nc.m.functions:
        for blk in fn.blocks:
            newlist = []
            for ins in blk.instructions:
                si = ins.sync_info
                n = len(si.on_wait) if si is not None else 0
                if n > 1:
                    waits = list(si.on_wait)
                    for j, w in enumerate(waits[:-1]):
                        nop = mybir.InstNoOp(
                            name=f"{ins.name}-hw{j}", engine=ins.engine,
                            ins=[], outs=[],
                            sync_info=bass_rust.SyncInfo(on_wait=[w],
                                                         on_update=[]))
                        nc.register_instruction(nop, overwrite=True)
                        newlist.append(nop)
                    si.on_wait = waits[-1:]
                newlist.append(ins)
            blk.instructions = newlist

B, S, D = 2, 2048, 4096
NQ, NKV, DK = 32, 4, 128
NCORES = 8
HPC = NQ // NCORES          # 4 q heads per core
DKC = HPC * DK              # 512 proj cols per core
SCALE = 1.0 / float(np.sqrt(DK))
MV = 30.0                   # masked logit magnitude (post-scale)
EXP_M = float(np.exp(-MV))
QB = 512                    # q block (matmul moving free dim)
KC = 128                    # k chunk (PE contraction/partition dim)
F32 = mybir.dt.float32
BF16 = mybir.dt.bfloat16
EXP = mybir.ActivationFunctionType.Exp


def build_program(s=S):
    """Build the per-core Bass/Tile program. Same program for all 8 cores
    (SPMD); per-core weight slices are supplied via the input maps."""
    nqb = s // QB            # q blocks
    nkc = s // KC            # k chunks
    nd = D // KC             # D contraction chunks (32)
    nnb = D // QB            # 8 column blocks of Wo

    nc = bass.Bass("TRN2", target_bir_lowering=False, debug=False,
                   num_devices=NCORES)
    xb = nc.dram_tensor("xb", [B, s, D], BF16, kind="ExternalInput").ap()
    wq = nc.dram_tensor("wq", [D, DKC], BF16, kind="ExternalInput").ap()
    wk = nc.dram_tensor("wk", [D, DK], BF16, kind="ExternalInput").ap()
    wv = nc.dram_tensor("wv", [D, DK], BF16, kind="ExternalInput").ap()
    wo = nc.dram_tensor("wo", [DKC, D], BF16, kind="ExternalInput").ap()
    m01 = nc.dram_tensor("mask01", [4, KC, QB], BF16, kind="ExternalInput").ap()
    mem = nc.dram_tensor("maskem", [4, KC, QB], BF16, kind="ExternalInput").ap()
    out = nc.dram_tensor("out", [B, s, D], F32, kind="ExternalOutput").ap()

    xf = xb.rearrange("b s d -> (b s) d")
    of = out.rearrange("b s d -> (b s) d")

    with tile.TileContext(nc) as tc, ExitStack() as ctx:
        consts = ctx.enter_context(tc.tile_pool(name="consts", bufs=1))
        ident = consts.tile([128, 128], BF16, name="ident", tag="ident")
        make_identity(nc, ident)
        ones = consts.tile([128, 128], BF16, name="ones", tag="ones")
        nc.vector.memset(ones, 1.0)

        # masks (bf16, applied post-exp)
        m01_t = consts.tile([128, 4, QB], BF16, name="m01_t", tag="m01_t")
        nc.sync.dma_start(out=m01_t, in_=m01.rearrange("d p n -> p d n"))
        mem_t = consts.tile([128, 4, QB], BF16, name="mem_t", tag="mem_t")
        nc.sync.dma_start(out=mem_t, in_=mem.rearrange("d p n -> p d n"))

        # weights: loaded once, reused for both batches
        wpool = ctx.enter_context(tc.tile_pool(name="wqkv", bufs=1))
        wq_t = wpool.tile([128, nd, DKC], BF16, name="wq_t", tag="wq_t")
        nc.sync.dma_start(out=wq_t, in_=wq.rearrange("(c p) n -> p c n", p=128))
        wk_t = wpool.tile([128, nd, DK], BF16, name="wk_t", tag="wk_t")
        nc.sync.dma_start(out=wk_t, in_=wk.rearrange("(c p) n -> p c n", p=128))
        wv_t = wpool.tile([128, nd, DK], BF16, name="wv_t", tag="wv_t")
        nc.sync.dma_start(out=wv_t, in_=wv.rearrange("(c p) n -> p c n", p=128))
        wo_t = wpool.tile([128, HPC, nnb, QB], BF16, name="wo_t", tag="wo_t")
        nc.sync.dma_start(
            out=wo_t,
            in_=wo.rearrange("(c p) (nb n) -> p c nb n", p=128, n=QB))

        nskip = 4 * (nqb - 1)   # fully-masked chunks of the last q block

        for b in range(B):
            with ExitStack() as bctx:
                bpool = bctx.enter_context(tc.tile_pool(name=f"bp{b}", bufs=1))
                qt = [bpool.tile([128, s], BF16, name=f"qt{b}_{h}", tag=f"qt{h}")
                      for h in range(HPC)]
                kt = bpool.tile([128, s], BF16, name=f"kt{b}", tag="kt")
                vt = bpool.tile([128, s], BF16, name=f"vt{b}", tag="vt")
                vn = bpool.tile([128, s], BF16, name=f"vn{b}", tag="vn")
                cv = bpool.tile([128, 1], F32, name=f"cv{b}", tag="cv")

                # ---------- projection phase: Q^T, K^T, V^T ----------
                ndq = 4                  # x loaded in 4 column quarters
                dq = D // ndq            # 1024
                with ExitStack() as pctx:
                    xpool = pctx.enter_context(tc.tile_pool(name="xload", bufs=6))
                    xtp = pctx.enter_context(tc.tile_pool(name="xtsb", bufs=4))
                    ppool = pctx.enter_context(
                        tc.tile_pool(name="projpsum", bufs=1, space="PSUM"))
                    tpool = pctx.enter_context(
                        tc.tile_pool(name="trpsum", bufs=2, space="PSUM"))

                    for qb in range(nqb):
                        pq = [ppool.tile([128, QB], F32, name=f"pq{h}", tag=f"pq{h}")
                              for h in range(HPC)]
                        pk = ppool.tile([128, QB], F32, name="pk", tag="pk")
                        pv = ppool.tile([128, QB], F32, name="pv", tag="pv")
                        for dqi in range(ndq):
                            xts = []
                            for rt in range(4):
                                xt_ = xpool.tile([128, dq], BF16, name="xt", tag="xt")
                                row0 = b * s + qb * QB + rt * 128
                                nc.sync.dma_start(
                                    out=xt_,
                                    in_=xf[row0:row0 + 128, dqi * dq:(dqi + 1) * dq])
                                xts.append(xt_)
                            for kci in range(dq // KC):
                                dc = dqi * (dq // KC) + kci
                                ptp = tpool.tile([128, QB], BF16, name="ptp", tag="ptp")
                                for rt in range(4):
                                    nc.tensor.transpose(
                                        ptp[:, rt * 128:(rt + 1) * 128],
                                        xts[rt][:, kci * 128:(kci + 1) * 128],
                                        ident)
                                xT = xtp.tile([128, QB], BF16, name="xT", tag="xT")
                                if dc % 2 == 0:
                                    nc.vector.tensor_copy(xT, ptp)
                                else:
                                    nc.scalar.copy(xT, ptp)
                                st = dc == 0
                                sp = dc == nd - 1
                                for h in range(HPC):
                                    nc.tensor.matmul(
                                        pq[h], wq_t[:, dc, h * 128:(h + 1) * 128],
                                        xT, start=st, stop=sp)
                                nc.tensor.matmul(pk, wk_t[:, dc, :], xT,
                                                 start=st, stop=sp)
                                nc.tensor.matmul(pv, wv_t[:, dc, :], xT,
                                                 start=st, stop=sp)
                        sl = slice(qb * QB, (qb + 1) * QB)
                        for h in range(HPC):
                            nc.vector.tensor_copy(qt[h][:, sl], pq[h])
                        nc.vector.tensor_copy(kt[:, sl], pk)
                        nc.vector.tensor_copy(vt[:, sl], pv)

                # ---------- V^T -> V natural; cv = exp(-30)*cumsum(V) ------
                with ExitStack() as vctx:
                    vpsum = vctx.enter_context(
                        tc.tile_pool(name="vtpsum", bufs=2, space="PSUM"))
                    for kc in range(nkc):
                        pvt = vpsum.tile([128, 128], BF16, name="pvt", tag="pvt")
                        nc.tensor.transpose(
                            pvt, vt[:, kc * 128:(kc + 1) * 128], ident)
                        nc.vector.tensor_copy(vn[:, kc * 128:(kc + 1) * 128], pvt)
                    if nskip > 0:
                        cps = vctx.enter_context(
                            tc.tile_pool(name="cvpsum", bufs=1, space="PSUM"))
                        pc = cps.tile([128, 8], F32, name="pc", tag="pc")
                        for i in range(nskip):
                            nc.tensor.matmul(
                                pc, vn[:, i * 128:(i + 1) * 128], ones[:, 0:8],
                                start=(i == 0), stop=(i == nskip - 1))
                        nc.scalar.mul(cv, pc[:, 0:1], EXP_M)

                # ---------- attention ----------
                apool = bctx.enter_context(tc.tile_pool(name=f"att{b}", bufs=1))
                att = [apool.tile([128, s], BF16, name=f"att{b}_{h}", tag=f"att{h}")
                       for h in range(HPC)]
                with ExitStack() as actx:
                    aps = actx.enter_context(
                        tc.tile_pool(name="atpsum", bufs=2, space="PSUM"))
                    sps = actx.enter_context(
                        tc.tile_pool(name="scpsum", bufs=2, space="PSUM"))
                    spool = actx.enter_context(tc.tile_pool(name="attsb", bufs=3))
                    ptp2 = actx.enter_context(tc.tile_pool(name="ptsb", bufs=3))

                    for h in range(HPC):
                        for qb in range(nqb):
                            last = qb == nqb - 1
                            qsl = slice(qb * QB, (qb + 1) * QB)
                            kcs = list(range(4 * qb, nkc))
                            npair = len(kcs) // 2
                            po = aps.tile([128, QB], F32, name="po", tag="po")
                            pr = aps.tile([128, QB], F32, name="pr", tag="pr")
                            for pi in range(npair):
                                kc0 = kcs[2 * pi]
                                ps2 = sps.tile([128, 2 * QB], F32, name="ps2",
                                               tag="ps2")
                                for half in range(2):
                                    kc = kc0 + half
                                    hsl = slice(half * QB, (half + 1) * QB)
                                    nc.tensor.matmul(
                                        ps2[:, hsl],
                                        kt[:, kc * 128:(kc + 1) * 128],
                                        qt[h][:, qsl], start=True, stop=True)
                                pt2 = ptp2.tile([128, 2 * QB], BF16, name="pt2",
                                                tag="pt2")
                                nc.scalar.activation(pt2, ps2, EXP, scale=SCALE)
                                for half in range(2):
                                    kc = kc0 + half
                                    hsl = slice(half * QB, (half + 1) * QB)
                                    d = kc - 4 * qb
                                    if d < 4:
                                        nc.vector.tensor_mul(
                                            pt2[:, hsl], pt2[:, hsl],
                                            m01_t[:, d, :])
                                        if last:
                                            nc.vector.tensor_add(
                                                pt2[:, hsl], pt2[:, hsl],
                                                mem_t[:, d, :])
                                    i = 2 * pi + half
                                    nc.tensor.matmul(
                                        po, vn[:, kc * 128:(kc + 1) * 128],
                                        pt2[:, hsl],
                                        start=(i == 0), stop=(i == len(kcs) - 1))
                                    nc.tensor.matmul(
                                        pr, ones, pt2[:, hsl],
                                        start=(i == 0), stop=(i == len(kcs) - 1))
                            rr = spool.tile([128, QB], F32, name="rr", tag="rr")
                            if last and nskip > 0:
                                rbias = spool.tile([128, QB], F32, name="rbias",
                                                   tag="rbias")
                                nc.vector.tensor_scalar_add(
                                    rbias, pr, float(nskip * 128 * EXP_M))
                                nc.vector.reciprocal(rr, rbias)
                                tno = spool.tile([128, QB], F32, name="tno",
                                                 tag="tno")
                                nc.vector.tensor_scalar_add(tno, po, cv)
                                nc.vector.tensor_mul(att[h][:, qsl], tno, rr)
                            else:
                                nc.vector.reciprocal(rr, pr)
                                nc.vector.tensor_mul(att[h][:, qsl], po, rr)

                # ---------- output projection (partial: this core's heads) ----
                with ExitStack() as wctx:
                    opsum = wctx.enter_context(
                        tc.tile_pool(name="opsum", bufs=4, space="PSUM"))
                    stpool = wctx.enter_context(tc.tile_pool(name="ostage", bufs=2))
                    for qti in range(s // 128):
                        stg = stpool.tile([128, D], F32, name="stg", tag="stg")
                        for nb in range(nnb):
                            po2 = opsum.tile([128, QB], F32, name="po2", tag="po2")
                            for c in range(HPC):
                                nc.tensor.matmul(
                                    po2, att[c][:, qti * 128:(qti + 1) * 128],
                                    wo_t[:, c, nb, :],
                                    start=(c == 0), stop=(c == HPC - 1))
                            osl = slice(nb * QB, (nb + 1) * QB)
                            if nb % 2 == 0:
                                nc.vector.tensor_copy(stg[:, osl], po2)
                            else:
                                nc.scalar.copy(stg[:, osl], po2)
                        row0 = b * s + qti * 128
                        nc.sync.dma_start(out=of[row0:row0 + 128, :], in_=stg)
    _split_multiwaits(nc)
    return nc


def make_masks():
    import ml_dtypes

    bf = ml_dtypes.bfloat16
    r = np.arange(KC)[:, None]
    c = np.arange(QB)[None, :]
    valid = [(r + 128 * d) > c for d in range(4)]   # k > q within block
    m01 = np.stack([v.astype(np.float32) for v in valid]).astype(bf)
    mem = np.stack([np.where(v, 0.0, EXP_M) for v in valid]).astype(bf)
    return m01, mem


_PROG = {}


def _get_program(s=S):
    if s not in _PROG:
        _PROG[s] = build_program(s)
    return _PROG[s]


def core_in_map(c, x, Wq, Wk, Wv, Wo, _shared={}):
    import ml_dtypes

    bf = ml_dtypes.bfloat16
    xid = id(x)
    if _shared.get("xid") != xid:
        _shared["xid"] = xid
        _shared["xb"] = np.ascontiguousarray(
            np.asarray(x, dtype=np.float32).astype(bf))
        _shared["m01"], _shared["mem"] = make_masks()
    h0 = c * HPC
    kv = (c * HPC) // (NQ // NKV)
    return {
        "xb": _shared["xb"],
        "wq": np.ascontiguousarray(
            np.asarray(Wq, np.float32)[:, h0 * DK:(h0 + HPC) * DK].astype(bf)),
        "wk": np.ascontiguousarray(
            np.asarray(Wk, np.float32)[:, kv * DK:(kv + 1) * DK].astype(bf)),
        "wv": np.ascontiguousarray(
            np.asarray(Wv, np.float32)[:, kv * DK:(kv + 1) * DK].astype(bf)),
        "wo": np.ascontiguousarray(
            np.asarray(Wo, np.float32)[h0 * DK:(h0 + HPC) * DK, :].astype(bf)),
        "mask01": _shared["m01"],
        "maskem": _shared["mem"],
    }


def kernel(x, Wq, Wk, Wv, Wo, **kw):
    from concourse.bass_utils import run_bass_kernel_spmd

    nc = _get_program(np.asarray(x).shape[1])
    in_maps = [core_in_map(c, x, Wq, Wk, Wv, Wo) for c in range(NCORES)]
    res = run_bass_kernel_spmd(nc, in_maps, core_ids=list(range(NCORES)), **kw)
    acc = np.zeros(np.asarray(x).shape, np.float64)
    for r in res.results:
        acc += r["out"]
    return acc.astype(np.float32)


# revision 30
# speedup vs baseline: 1.1776x; 1.1776x over previous
"""Trainium2 Bass kernel for GroupedQueryAttention (anti-causal mask variant).

Reference semantics (B=2, S=2048, D=4096, 32 Q heads, 4 KV heads, dk=128):
  Q = x@Wq, K = x@Wk, V = x@Wv (heads split), GQA repeat KV x8.
  scores = Q K^T / sqrt(dk); mask = triu(ones, k=1); scores = where(mask==0, -1e9, scores)
    -> keeps STRICT UPPER triangle (k > q, anti-causal). Rows with no valid key
       (q == S-1) become a uniform softmax over all S keys.
  out = softmax(scores) @ V; out = out @ Wo.

Sharding: 8 cores, 4 Q heads + their 1 shared KV head per core. Each core
computes a partial out = attn_heads @ Wo_rows_slice; host sums the 8 partials.

Per-core kernel design (bf16 operands, fp32 PSUM accumulation):
  - x is pre-cast to bf16 on the host (inputs stay fp32 at the kernel()
    boundary); x^T tiles via PE transposes (bf16, 1 cycle/row) + DVE/ACT
    copies out of PSUM.
  - Q^T/K^T/V^T projections in [dk, seq] layout (lhsT = bf16 W chunk, FWL).
  - scores computed TRANSPOSED: sT[k, q] = K^T chunk (lhsT) x Q^T (rhs), so
    softmax denominator is a partition-dim sum (ones-matmul) and the AV matmul
    out^T[dk, q] = V chunk (lhsT) x P^T (rhs) accumulates with N=512 and lands
    already transposed for the Wo projection.
  - exp on ACT over CHUNK PAIRS ([128,1024] spanning two PSUM banks), bf16
    out; masking applied POST-exp as cheap bf16 multiplies on the DVE
    (pt *= M01 gives exact zeros, matching exp(-1e9) -> 0). For the LAST q
    block the reference's fully-masked rows need uniform weights, so there
    pt = exp(s)*M01 + exp(-30)*(1-M01), and the skipped blocks' contributions
    are added analytically: r += n_skip*128*exp(-30), out^T += exp(-30)*cumsumV.
"""

import sys
from contextlib import ExitStack

import numpy as np

for _p in ("/opt/trn_rl_repo",):
    if _p not in sys.path:
        sys.path.insert(0, _p)

import bass_rust
import concourse.bass as bass
import concourse.mybir as mybir
import concourse.tile as tile
from concourse.masks import make_identity


def _split_multiwaits(nc):
    """This walrus build encodes at most ONE sem wait per instruction.
    Tile's wait-assignment can attach several; hoist the extras onto fresh
    single-wait NoOps emitted immediately before the instruction on the same
    engine stream. Tile emits instructions in schedule order, so every wait's
    producer precedes the waiting instruction in-stream and the stall cannot
    deadlock."""
    for fn in nc.m.functions:
        for blk in fn.blocks:
            newlist = []
            for ins in blk.instructions:
                si = ins.sync_info
                n = len(si.on_wait) if si is not None else 0
                if n > 1:
                    waits = list(si.on_wait)
                    for j, w in enumerate(waits[:-1]):
                        nop = mybir.InstNoOp(
                            name=f"{ins.name}-hw{j}", engine=ins.engine,
                            ins=[], outs=[],
                            sync_info=bass_rust.SyncInfo(on_wait=[w],
                                                         on_update=[]))
                        nc.register_instruction(nop, overwrite=True)
                        newlist.append(nop)
                    si.on_wait = waits[-1:]
                newlist.append(ins)
            blk.instructions = newlist

B, S, D = 2, 2048, 4096
NQ, NKV, DK = 32, 4, 128
NCORES = 8
HPC = NQ // NCORES          # 4 q heads per core
DKC = HPC * DK              # 512 proj cols per core
SCALE = 1.0 / float(np.sqrt(DK))
MV = 30.0                   # masked logit magnitude (post-scale)
EXP_M = float(np.exp(-MV))
QB = 512                    # q block (matmul moving free dim)
KC = 128                    # k chunk (PE contraction/partition dim)
F32 = mybir.dt.float32
BF16 = mybir.dt.bfloat16
EXP = mybir.ActivationFunctionType.Exp


def build_program(s=S):
    """Build the per-core Bass/Tile program. Same program for all 8 cores
    (SPMD); per-core weight slices are supplied via the input maps."""
    nqb = s // QB            # q blocks
    nkc = s // KC            # k chunks
    nd = D // KC             # D contraction chunks (32)
    nnb = D // QB            # 8 column blocks of Wo

    nc = bass.Bass("TRN2", target_bir_lowering=False, debug=False,
                   num_devices=NCORES)
    xb = nc.dram_tensor("xb", [B, s, D], BF16, kind="ExternalInput").ap()
    wq = nc.dram_tensor("wq", [D, DKC], BF16, kind="ExternalInput").ap()
    wk = nc.dram_tensor("wk", [D, DK], BF16, kind="ExternalInput").ap()
    wv = nc.dram_tensor("wv", [D, DK], BF16, kind="ExternalInput").ap()
    wo = nc.dram_tensor("wo", [DKC, D], BF16, kind="ExternalInput").ap()
    m01 = nc.dram_tensor("mask01", [4, KC, QB], BF16, kind="ExternalInput").ap()
    mem = nc.dram_tensor("maskem", [4, KC, QB], BF16, kind="ExternalInput").ap()
    out = nc.dram_tensor("out", [B, s, D], F32, kind="ExternalOutput").ap()

    xf = xb.rearrange("b s d -> (b s) d")
    of = out.rearrange("b s d -> (b s) d")

    with tile.TileContext(nc) as tc, ExitStack() as ctx:
        consts = ctx.enter_context(tc.tile_pool(name="consts", bufs=1))
        ident = consts.tile([128, 128], BF16, name="ident", tag="ident")
        make_identity(nc, ident)
        ones = consts.tile([128, 128], BF16, name="ones", tag="ones")
        nc.vector.memset(ones, 1.0)

        # masks (bf16, applied post-exp)
        m01_t = consts.tile([128, 4, QB], BF16, name="m01_t", tag="m01_t")
        nc.sync.dma_start(out=m01_t, in_=m01.rearrange("d p n -> p d n"))
        mem_t = consts.tile([128, 4, QB], BF16, name="mem_t", tag="mem_t")
        nc.sync.dma_start(out=mem_t, in_=mem.rearrange("d p n -> p d n"))

        # weights: loaded once, reused for both batches
        wpool = ctx.enter_context(tc.tile_pool(name="wqkv", bufs=1))
        wq_t = wpool.tile([128, nd, DKC], BF16, name="wq_t", tag="wq_t")
        nc.sync.dma_start(out=wq_t, in_=wq.rearrange("(c p) n -> p c n", p=128))
        wk_t = wpool.tile([128, nd, DK], BF16, name="wk_t", tag="wk_t")
        nc.sync.dma_start(out=wk_t, in_=wk.rearrange("(c p) n -> p c n", p=128))
        wv_t = wpool.tile([128, nd, DK], BF16, name="wv_t", tag="wv_t")
        nc.sync.dma_start(out=wv_t, in_=wv.rearrange("(c p) n -> p c n", p=128))
        wo_t = wpool.tile([128, HPC, nnb, QB], BF16, name="wo_t", tag="wo_t")
        nc.sync.dma_start(
            out=wo_t,
            in_=wo.rearrange("(c p) (nb n) -> p c nb n", p=128, n=QB))

        nskip = 4 * (nqb - 1)   # fully-masked chunks of the last q block

        for b in range(B):
            with ExitStack() as bctx:
                bpool = bctx.enter_context(tc.tile_pool(name=f"bp{b}", bufs=1))
                qt = [bpool.tile([128, s], BF16, name=f"qt{b}_{h}", tag=f"qt{h}")
                      for h in range(HPC)]
                kt = bpool.tile([128, s], BF16, name=f"kt{b}", tag="kt")
                vt = bpool.tile([128, s], BF16, name=f"vt{b}", tag="vt")
                vn = bpool.tile([128, s], BF16, name=f"vn{b}", tag="vn")
                cv = bpool.tile([128, 1], F32, name=f"cv{b}", tag="cv")

                # ---------- projection phase: Q^T, K^T, V^T ----------
                ndq = 4                  # x loaded in 4 column quarters
                dq = D // ndq            # 1024
                with ExitStack() as pctx:
                    xpool = pctx.enter_context(tc.tile_pool(name="xload", bufs=8))
                    xtp = pctx.enter_context(tc.tile_pool(name="xtsb", bufs=4))
                    ppool = pctx.enter_context(
                        tc.tile_pool(name="projpsum", bufs=1, space="PSUM"))
                    tpool = pctx.enter_context(
                        tc.tile_pool(name="trpsum", bufs=2, space="PSUM"))

                    for qb in range(nqb):
                        pq = [ppool.tile([128, QB], F32, name=f"pq{h}", tag=f"pq{h}")
                              for h in range(HPC)]
                        pk = ppool.tile([128, QB], F32, name="pk", tag="pk")
                        pv = ppool.tile([128, QB], F32, name="pv", tag="pv")
                        for dqi in range(ndq):
                            xts = []
                            for rt in range(4):
                                xt_ = xpool.tile([128, dq], BF16, name="xt", tag="xt")
                                row0 = b * s + qb * QB + rt * 128
                                nc.sync.dma_start(
                                    out=xt_,
                                    in_=xf[row0:row0 + 128, dqi * dq:(dqi + 1) * dq])
                                xts.append(xt_)
                            for kci in range(dq // KC):
                                dc = dqi * (dq // KC) + kci
                                ptp = tpool.tile([128, QB], BF16, name="ptp", tag="ptp")
                                for rt in range(4):
                                    nc.tensor.transpose(
                                        ptp[:, rt * 128:(rt + 1) * 128],
                                        xts[rt][:, kci * 128:(kci + 1) * 128],
                                        ident)
                                xT = xtp.tile([128, QB], BF16, name="xT", tag="xT")
                                if dc % 2 == 0:
                                    nc.vector.tensor_copy(xT, ptp)
                                else:
                                    nc.scalar.copy(xT, ptp)
                                st = dc == 0
                                sp = dc == nd - 1
                                for h in range(HPC):
                                    nc.tensor.matmul(
                                        pq[h], wq_t[:, dc, h * 128:(h + 1) * 128],
                                        xT, start=st, stop=sp)
                                nc.tensor.matmul(pk, wk_t[:, dc, :], xT,
                                                 start=st, stop=sp)
                                nc.tensor.matmul(pv, wv_t[:, dc, :], xT,
                                                 start=st, stop=sp)
                        sl = slice(qb * QB, (qb + 1) * QB)
                        for h in range(HPC):
                            nc.vector.tensor_copy(qt[h][:, sl], pq[h])
                        nc.vector.tensor_copy(kt[:, sl], pk)
                        nc.vector.tensor_copy(vt[:, sl], pv)

                # ---------- V^T -> V natural; cv = exp(-30)*cumsum(V) ------
                with ExitStack() as vctx:
                    vpsum = vctx.enter_context(
                        tc.tile_pool(name="vtpsum", bufs=2, space="PSUM"))
                    for kc in range(nkc):
                        pvt = vpsum.tile([128, 128], BF16, name="pvt", tag="pvt")
                        nc.tensor.transpose(
                            pvt, vt[:, kc * 128:(kc + 1) * 128], ident)
                        nc.vector.tensor_copy(vn[:, kc * 128:(kc + 1) * 128], pvt)
                    if nskip > 0:
                        cps = vctx.enter_context(
                            tc.tile_pool(name="cvpsum", bufs=1, space="PSUM"))
                        pc = cps.tile([128, 8], F32, name="pc", tag="pc")
                        for i in range(nskip):
                            nc.tensor.matmul(
                                pc, vn[:, i * 128:(i + 1) * 128], ones[:, 0:8],
                                start=(i == 0), stop=(i == nskip - 1))
                        nc.scalar.mul(cv, pc[:, 0:1], EXP_M)

                # ---------- attention ----------
                apool = bctx.enter_context(tc.tile_pool(name=f"att{b}", bufs=1))
                att = [apool.tile([128, s], BF16, name=f"att{b}_{h}", tag=f"att{h}")
                       for h in range(HPC)]
                with ExitStack() as actx:
                    aps = actx.enter_context(
                        tc.tile_pool(name="atpsum", bufs=2, space="PSUM"))
                    sps = actx.enter_context(
                        tc.tile_pool(name="scpsum", bufs=2, space="PSUM"))
                    spool = actx.enter_context(tc.tile_pool(name="attsb", bufs=3))
                    ptp2 = actx.enter_context(tc.tile_pool(name="ptsb", bufs=3))

                    for h in range(HPC):
                        for qb in range(nqb):
                            last = qb == nqb - 1
                            qsl = slice(qb * QB, (qb + 1) * QB)
                            kcs = list(range(4 * qb, nkc))
                            npair = len(kcs) // 2
                            po = aps.tile([128, QB], F32, name="po", tag="po")
                            pr = aps.tile([128, QB], F32, name="pr", tag="pr")
                            for pi in range(npair):
                                kc0 = kcs[2 * pi]
                                ps2 = sps.tile([128, 2 * QB], F32, name="ps2",
                                               tag="ps2")
                                for half in range(2):
                                    kc = kc0 + half
                                    hsl = slice(half * QB, (half + 1) * QB)
                                    nc.tensor.matmul(
                                        ps2[:, hsl],
                                        kt[:, kc * 128:(kc + 1) * 128],
                                        qt[h][:, qsl], start=True, stop=True)
                                pt2 = ptp2.tile([128, 2 * QB], BF16, name="pt2",
                                                tag="pt2")
                                nc.scalar.activation(pt2, ps2, EXP, scale=SCALE)
                                for half in range(2):
                                    kc = kc0 + half
                                    hsl = slice(half * QB, (half + 1) * QB)
                                    d = kc - 4 * qb
                                    if d < 4:
                                        # POOL: keeps DVE free for reciprocal
                                        nc.gpsimd.tensor_mul(
                                            pt2[:, hsl], pt2[:, hsl],
                                            m01_t[:, d, :])
                                        if last:
                                            nc.gpsimd.tensor_add(
                                                pt2[:, hsl], pt2[:, hsl],
                                                mem_t[:, d, :])
                                    i = 2 * pi + half
                                    nc.tensor.matmul(
                                        po, vn[:, kc * 128:(kc + 1) * 128],
                                        pt2[:, hsl],
                                        start=(i == 0), stop=(i == len(kcs) - 1))
                                    nc.tensor.matmul(
                                        pr, ones, pt2[:, hsl],
                                        start=(i == 0), stop=(i == len(kcs) - 1))
                            rr = spool.tile([128, QB], F32, name="rr", tag="rr")
                            if last and nskip > 0:
                                rbias = spool.tile([128, QB], F32, name="rbias",
                                                   tag="rbias")
                                nc.vector.tensor_scalar_add(
                                    rbias, pr, float(nskip * 128 * EXP_M))
                                nc.vector.reciprocal(rr, rbias)
                                tno = spool.tile([128, QB], F32, name="tno",
                                                 tag="tno")
                                nc.vector.tensor_scalar_add(tno, po, cv)
                                nc.vector.tensor_mul(att[h][:, qsl], tno, rr)
                            else:
                                nc.vector.reciprocal(rr, pr)
                                nc.vector.tensor_mul(att[h][:, qsl], po, rr)

                # ---------- output projection (partial: this core's heads) ----
                with ExitStack() as wctx:
                    opsum = wctx.enter_context(
                        tc.tile_pool(name="opsum", bufs=4, space="PSUM"))
                    stpool = wctx.enter_context(tc.tile_pool(name="ostage", bufs=2))
                    for qti in range(s // 128):
                        stg = stpool.tile([128, D], F32, name="stg", tag="stg")
                        for nb in range(nnb):
                            po2 = opsum.tile([128, QB], F32, name="po2", tag="po2")
                            for c in range(HPC):
                                nc.tensor.matmul(
                                    po2, att[c][:, qti * 128:(qti + 1) * 128],
                                    wo_t[:, c, nb, :],
                                    start=(c == 0), stop=(c == HPC - 1))
                            osl = slice(nb * QB, (nb + 1) * QB)
                            if nb % 2 == 0:
                                nc.vector.tensor_copy(stg[:, osl], po2)
                            else:
                                nc.scalar.copy(stg[:, osl], po2)
                        row0 = b * s + qti * 128
                        nc.sync.dma_start(out=of[row0:row0 + 128, :], in_=stg)
    _split_multiwaits(nc)
    return nc


def make_masks():
    import ml_dtypes

    bf = ml_dtypes.bfloat16
    r = np.arange(KC)[:, None]
    c = np.arange(QB)[None, :]
    valid = [(r + 128 * d) > c for d in range(4)]   # k > q within block
    m01 = np.stack([v.astype(np.float32) for v in valid]).astype(bf)
    mem = np.stack([np.where(v, 0.0, EXP_M) for v in valid]).astype(bf)
    return m01, mem


_PROG = {}


def _get_program(s=S):
    if s not in _PROG:
        _PROG[s] = build_program(s)
    return _PROG[s]


def core_in_map(c, x, Wq, Wk, Wv, Wo, _shared={}):
    import ml_dtypes

    bf = ml_dtypes.bfloat16
    xid = id(x)
    if _shared.get("xid") != xid:
        _shared["xid"] = xid
        _shared["xb"] = np.ascontiguousarray(
            np.asarray(x, dtype=np.float32).astype(bf))
        _shared["m01"], _shared["mem"] = make_masks()
    h0 = c * HPC
    kv = (c * HPC) // (NQ // NKV)
    return {
        "xb": _shared["xb"],
        "wq": np.ascontiguousarray(
            np.asarray(Wq, np.float32)[:, h0 * DK:(h0 + HPC) * DK].astype(bf)),
        "wk": np.ascontiguousarray(
            np.asarray(Wk, np.float32)[:, kv * DK:(kv + 1) * DK].astype(bf)),
        "wv": np.ascontiguousarray(
            np.asarray(Wv, np.float32)[:, kv * DK:(kv + 1) * DK].astype(bf)),
        "wo": np.ascontiguousarray(
            np.asarray(Wo, np.float32)[h0 * DK:(h0 + HPC) * DK, :].astype(bf)),
        "mask01": _shared["m01"],
        "maskem": _shared["mem"],
    }


def kernel(x, Wq, Wk, Wv, Wo, **kw):
    from concourse.bass_utils import run_bass_kernel_spmd

    nc = _get_program(np.asarray(x).shape[1])
    in_maps = [core_in_map(c, x, Wq, Wk, Wv, Wo) for c in range(NCORES)]
    res = run_bass_kernel_spmd(nc, in_maps, core_ids=list(range(NCORES)), **kw)
    acc = np.zeros(np.asarray(x).shape, np.float64)
    for r in res.results:
        acc += r["out"]
    return acc.astype(np.float32)


# revision 31
# speedup vs baseline: 1.2298x; 1.0443x over previous
"""Trainium2 Bass kernel for GroupedQueryAttention (anti-causal mask variant).

Reference semantics (B=2, S=2048, D=4096, 32 Q heads, 4 KV heads, dk=128):
  Q = x@Wq, K = x@Wk, V = x@Wv (heads split), GQA repeat KV x8.
  scores = Q K^T / sqrt(dk); mask = triu(ones, k=1); scores = where(mask==0, -1e9, scores)
    -> keeps STRICT UPPER triangle (k > q, anti-causal). Rows with no valid key
       (q == S-1) become a uniform softmax over all S keys.
  out = softmax(scores) @ V; out = out @ Wo.

Sharding: 8 cores, 4 Q heads + their 1 shared KV head per core. Each core
computes a partial out = attn_heads @ Wo_rows_slice; host sums the 8 partials.

Per-core kernel design (bf16 operands, fp32 PSUM accumulation):
  - x is pre-cast to bf16 on the host (inputs stay fp32 at the kernel()
    boundary); x^T tiles via PE transposes (bf16, 1 cycle/row) + DVE/ACT
    copies out of PSUM.
  - Q^T/K^T/V^T projections in [dk, seq] layout (lhsT = bf16 W chunk, FWL).
  - scores computed TRANSPOSED: sT[k, q] = K^T chunk (lhsT) x Q^T (rhs), so
    softmax denominator is a partition-dim sum (ones-matmul) and the AV matmul
    out^T[dk, q] = V chunk (lhsT) x P^T (rhs) accumulates with N=512 and lands
    already transposed for the Wo projection.
  - exp on ACT over CHUNK PAIRS ([128,1024] spanning two PSUM banks), bf16
    out; masking applied POST-exp as cheap bf16 multiplies on the DVE
    (pt *= M01 gives exact zeros, matching exp(-1e9) -> 0). For the LAST q
    block the reference's fully-masked rows need uniform weights, so there
    pt = exp(s)*M01 + exp(-30)*(1-M01), and the skipped blocks' contributions
    are added analytically: r += n_skip*128*exp(-30), out^T += exp(-30)*cumsumV.
"""

import sys
from contextlib import ExitStack

import numpy as np

for _p in ("/opt/trn_rl_repo",):
    if _p not in sys.path:
        sys.path.insert(0, _p)

import bass_rust
import concourse.bass as bass
import concourse.mybir as mybir
import concourse.tile as tile
from concourse.masks import make_identity


def _split_multiwaits(nc):
    """This walrus build encodes at most ONE sem wait per instruction.
    Tile's wait-assignment can attach several; hoist the extras onto fresh
    single-wait NoOps emitted immediately before the instruction on the same
    engine stream. Tile emits instructions in schedule order, so every wait's
    producer precedes the waiting instruction in-stream and the stall cannot
    deadlock."""
    for fn in nc.m.functions:
        for blk in fn.blocks:
            newlist = []
            for ins in blk.instructions:
                si = ins.sync_info
                n = len(si.on_wait) if si is not None else 0
                if n > 1:
                    waits = list(si.on_wait)
                    for j, w in enumerate(waits[:-1]):
                        nop = mybir.InstNoOp(
                            name=f"{ins.name}-hw{j}", engine=ins.engine,
                            ins=[], outs=[],
                            sync_info=bass_rust.SyncInfo(on_wait=[w],
                                                         on_update=[]))
                        nc.register_instruction(nop, overwrite=True)
                        newlist.append(nop)
                    si.on_wait = waits[-1:]
                newlist.append(ins)
            blk.instructions = newlist

B, S, D = 2, 2048, 4096
NQ, NKV, DK = 32, 4, 128
NCORES = 8
HPC = NQ // NCORES          # 4 q heads per core
DKC = HPC * DK              # 512 proj cols per core
SCALE = 1.0 / float(np.sqrt(DK))
MV = 30.0                   # masked logit magnitude (post-scale)
EXP_M = float(np.exp(-MV))
QB = 512                    # q block (matmul moving free dim)
KC = 128                    # k chunk (PE contraction/partition dim)
F32 = mybir.dt.float32
BF16 = mybir.dt.bfloat16
EXP = mybir.ActivationFunctionType.Exp


def build_program(s=S):
    """Build the per-core Bass/Tile program. Same program for all 8 cores
    (SPMD); per-core weight slices are supplied via the input maps."""
    nqb = s // QB            # q blocks
    nkc = s // KC            # k chunks
    nd = D // KC             # D contraction chunks (32)
    nnb = D // QB            # 8 column blocks of Wo

    nc = bass.Bass("TRN2", target_bir_lowering=False, debug=False,
                   num_devices=NCORES)
    xb = nc.dram_tensor("xb", [B, s, D], BF16, kind="ExternalInput").ap()
    wq = nc.dram_tensor("wq", [D, DKC], BF16, kind="ExternalInput").ap()
    wk = nc.dram_tensor("wk", [D, DK], BF16, kind="ExternalInput").ap()
    wv = nc.dram_tensor("wv", [D, DK], BF16, kind="ExternalInput").ap()
    wo = nc.dram_tensor("wo", [DKC, D], BF16, kind="ExternalInput").ap()
    m01 = nc.dram_tensor("mask01", [4, KC, QB], BF16, kind="ExternalInput").ap()
    mem = nc.dram_tensor("maskem", [4, KC, QB], BF16, kind="ExternalInput").ap()
    out = nc.dram_tensor("out", [B, s, D], F32, kind="ExternalOutput").ap()

    xf = xb.rearrange("b s d -> (b s) d")
    of = out.rearrange("b s d -> (b s) d")

    with tile.TileContext(nc) as tc, ExitStack() as ctx:
        consts = ctx.enter_context(tc.tile_pool(name="consts", bufs=1))
        ident = consts.tile([128, 128], BF16, name="ident", tag="ident")
        make_identity(nc, ident)
        ones = consts.tile([128, 128], BF16, name="ones", tag="ones")
        nc.vector.memset(ones, 1.0)

        # masks (bf16, applied post-exp)
        m01_t = consts.tile([128, 4, QB], BF16, name="m01_t", tag="m01_t")
        nc.sync.dma_start(out=m01_t, in_=m01.rearrange("d p n -> p d n"))
        mem_t = consts.tile([128, 4, QB], BF16, name="mem_t", tag="mem_t")
        nc.sync.dma_start(out=mem_t, in_=mem.rearrange("d p n -> p d n"))

        # weights: loaded once, reused for both batches
        wpool = ctx.enter_context(tc.tile_pool(name="wqkv", bufs=1))
        wq_t = wpool.tile([128, nd, DKC], BF16, name="wq_t", tag="wq_t")
        nc.sync.dma_start(out=wq_t, in_=wq.rearrange("(c p) n -> p c n", p=128))
        wk_t = wpool.tile([128, nd, DK], BF16, name="wk_t", tag="wk_t")
        nc.sync.dma_start(out=wk_t, in_=wk.rearrange("(c p) n -> p c n", p=128))
        wv_t = wpool.tile([128, nd, DK], BF16, name="wv_t", tag="wv_t")
        nc.sync.dma_start(out=wv_t, in_=wv.rearrange("(c p) n -> p c n", p=128))
        wo_t = wpool.tile([128, HPC, nnb, QB], BF16, name="wo_t", tag="wo_t")
        nc.sync.dma_start(
            out=wo_t,
            in_=wo.rearrange("(c p) (nb n) -> p c nb n", p=128, n=QB))

        nskip = 4 * (nqb - 1)   # fully-masked chunks of the last q block

        for b in range(B):
            with ExitStack() as bctx:
                bpool = bctx.enter_context(tc.tile_pool(name=f"bp{b}", bufs=1))
                qt = [bpool.tile([128, s], BF16, name=f"qt{b}_{h}", tag=f"qt{h}")
                      for h in range(HPC)]
                kt = bpool.tile([128, s], BF16, name=f"kt{b}", tag="kt")
                vt = bpool.tile([128, s], BF16, name=f"vt{b}", tag="vt")
                vn = bpool.tile([128, s], BF16, name=f"vn{b}", tag="vn")
                cv = bpool.tile([128, 1], F32, name=f"cv{b}", tag="cv")

                # ---------- projection phase: Q^T, K^T, V^T ----------
                ndq = 4                  # x loaded in 4 column quarters
                dq = D // ndq            # 1024
                with ExitStack() as pctx:
                    xpool = pctx.enter_context(tc.tile_pool(name="xload", bufs=8))
                    xtp = pctx.enter_context(tc.tile_pool(name="xtsb", bufs=4))
                    ppool = pctx.enter_context(
                        tc.tile_pool(name="projpsum", bufs=1, space="PSUM"))
                    tpool = pctx.enter_context(
                        tc.tile_pool(name="trpsum", bufs=2, space="PSUM"))

                    for qb in range(nqb):
                        pq = [ppool.tile([128, QB], F32, name=f"pq{h}", tag=f"pq{h}")
                              for h in range(HPC)]
                        pk = ppool.tile([128, QB], F32, name="pk", tag="pk")
                        pv = ppool.tile([128, QB], F32, name="pv", tag="pv")
                        for dqi in range(ndq):
                            xts = []
                            for rt in range(4):
                                xt_ = xpool.tile([128, dq], BF16, name="xt", tag="xt")
                                row0 = b * s + qb * QB + rt * 128
                                nc.sync.dma_start(
                                    out=xt_,
                                    in_=xf[row0:row0 + 128, dqi * dq:(dqi + 1) * dq])
                                xts.append(xt_)
                            for kci in range(dq // KC):
                                dc = dqi * (dq // KC) + kci
                                ptp = tpool.tile([128, QB], BF16, name="ptp", tag="ptp")
                                for rt in range(4):
                                    nc.tensor.transpose(
                                        ptp[:, rt * 128:(rt + 1) * 128],
                                        xts[rt][:, kci * 128:(kci + 1) * 128],
                                        ident)
                                xT = xtp.tile([128, QB], BF16, name="xT", tag="xT")
                                if dc % 2 == 0:
                                    nc.vector.tensor_copy(xT, ptp)
                                else:
                                    nc.scalar.copy(xT, ptp)
                                st = dc == 0
                                sp = dc == nd - 1
                                for h in range(HPC):
                                    nc.tensor.matmul(
                                        pq[h], wq_t[:, dc, h * 128:(h + 1) * 128],
                                        xT, start=st, stop=sp)
                                nc.tensor.matmul(pk, wk_t[:, dc, :], xT,
                                                 start=st, stop=sp)
                                nc.tensor.matmul(pv, wv_t[:, dc, :], xT,
                                                 start=st, stop=sp)
                        sl = slice(qb * QB, (qb + 1) * QB)
                        for h in range(HPC):
                            nc.vector.tensor_copy(qt[h][:, sl], pq[h])
                        nc.vector.tensor_copy(kt[:, sl], pk)
                        nc.vector.tensor_copy(vt[:, sl], pv)

                # ---------- V^T -> V natural; cv = exp(-30)*cumsum(V) ------
                with ExitStack() as vctx:
                    vpsum = vctx.enter_context(
                        tc.tile_pool(name="vtpsum", bufs=2, space="PSUM"))
                    for kc in range(nkc):
                        pvt = vpsum.tile([128, 128], BF16, name="pvt", tag="pvt")
                        nc.tensor.transpose(
                            pvt, vt[:, kc * 128:(kc + 1) * 128], ident)
                        nc.vector.tensor_copy(vn[:, kc * 128:(kc + 1) * 128], pvt)
                    if nskip > 0:
                        cps = vctx.enter_context(
                            tc.tile_pool(name="cvpsum", bufs=1, space="PSUM"))
                        pc = cps.tile([128, 8], F32, name="pc", tag="pc")
                        for i in range(nskip):
                            nc.tensor.matmul(
                                pc, vn[:, i * 128:(i + 1) * 128], ones[:, 0:8],
                                start=(i == 0), stop=(i == nskip - 1))
                        nc.scalar.mul(cv, pc[:, 0:1], EXP_M)

                # ---------- attention ----------
                apool = bctx.enter_context(tc.tile_pool(name=f"att{b}", bufs=1))
                att = [apool.tile([128, s], BF16, name=f"att{b}_{h}", tag=f"att{h}")
                       for h in range(HPC)]
                with ExitStack() as actx:
                    aps = actx.enter_context(
                        tc.tile_pool(name="atpsum", bufs=2, space="PSUM"))
                    sps = actx.enter_context(
                        tc.tile_pool(name="scpsum", bufs=2, space="PSUM"))
                    spool = actx.enter_context(tc.tile_pool(name="attsb", bufs=3))
                    ptp2 = actx.enter_context(tc.tile_pool(name="ptsb", bufs=3))

                    for h in range(HPC):
                        for qb in range(nqb):
                            last = qb == nqb - 1
                            qsl = slice(qb * QB, (qb + 1) * QB)
                            kcs = list(range(4 * qb, nkc))
                            npair = len(kcs) // 2
                            po = aps.tile([128, QB], F32, name="po", tag="po")
                            pr = aps.tile([128, QB], F32, name="pr", tag="pr")
                            for pi in range(npair):
                                kc0 = kcs[2 * pi]
                                ps2 = sps.tile([128, 2 * QB], F32, name="ps2",
                                               tag="ps2")
                                for half in range(2):
                                    kc = kc0 + half
                                    hsl = slice(half * QB, (half + 1) * QB)
                                    nc.tensor.matmul(
                                        ps2[:, hsl],
                                        kt[:, kc * 128:(kc + 1) * 128],
                                        qt[h][:, qsl], start=True, stop=True)
                                pt2 = ptp2.tile([128, 2 * QB], BF16, name="pt2",
                                                tag="pt2")
                                nc.scalar.activation(pt2, ps2, EXP, scale=SCALE)
                                for half in range(2):
                                    kc = kc0 + half
                                    hsl = slice(half * QB, (half + 1) * QB)
                                    d = kc - 4 * qb
                                    if d < 4:
                                        # POOL: keeps DVE free for reciprocal
                                        nc.gpsimd.tensor_mul(
                                            pt2[:, hsl], pt2[:, hsl],
                                            m01_t[:, d, :])
                                        if last:
                                            nc.gpsimd.tensor_add(
                                                pt2[:, hsl], pt2[:, hsl],
                                                mem_t[:, d, :])
                                    i = 2 * pi + half
                                    nc.tensor.matmul(
                                        po, vn[:, kc * 128:(kc + 1) * 128],
                                        pt2[:, hsl],
                                        start=(i == 0), stop=(i == len(kcs) - 1))
                                    nc.tensor.matmul(
                                        pr, ones, pt2[:, hsl],
                                        start=(i == 0), stop=(i == len(kcs) - 1))
                            rr = spool.tile([128, QB], F32, name="rr", tag="rr")
                            if last and nskip > 0:
                                rbias = spool.tile([128, QB], F32, name="rbias",
                                                   tag="rbias")
                                nc.vector.tensor_scalar_add(
                                    rbias, pr, float(nskip * 128 * EXP_M))
                                nc.vector.reciprocal(rr, rbias)
                                tno = spool.tile([128, QB], F32, name="tno",
                                                 tag="tno")
                                nc.vector.tensor_scalar_add(tno, po, cv)
                                nc.vector.tensor_mul(att[h][:, qsl], tno, rr)
                            else:
                                nc.vector.reciprocal(rr, pr)
                                nc.vector.tensor_mul(att[h][:, qsl], po, rr)

                # ---------- output projection (partial: this core's heads) ----
                with ExitStack() as wctx:
                    opsum = wctx.enter_context(
                        tc.tile_pool(name="opsum", bufs=2, space="PSUM"))
                    stpool = wctx.enter_context(tc.tile_pool(name="ostage", bufs=2))
                    for qti in range(s // 128):
                        stg = stpool.tile([128, D], F32, name="stg", tag="stg")
                        for nb in range(nnb):
                            po2 = opsum.tile([128, QB], F32, name="po2", tag="po2")
                            for c in range(HPC):
                                nc.tensor.matmul(
                                    po2, att[c][:, qti * 128:(qti + 1) * 128],
                                    wo_t[:, c, nb, :],
                                    start=(c == 0), stop=(c == HPC - 1))
                            osl = slice(nb * QB, (nb + 1) * QB)
                            if nb % 2 == 0:
                                nc.vector.tensor_copy(stg[:, osl], po2)
                            else:
                                nc.scalar.copy(stg[:, osl], po2)
                        row0 = b * s + qti * 128
                        nc.sync.dma_start(out=of[row0:row0 + 128, :], in_=stg)
    _split_multiwaits(nc)
    return nc


def make_masks():
    import ml_dtypes

    bf = ml_dtypes.bfloat16
    r = np.arange(KC)[:, None]
    c = np.arange(QB)[None, :]
    valid = [(r + 128 * d) > c for d in range(4)]   # k > q within block
    m01 = np.stack([v.astype(np.float32) for v in valid]).astype(bf)
    mem = np.stack([np.where(v, 0.0, EXP_M) for v in valid]).astype(bf)
    return m01, mem


_PROG = {}


def _get_program(s=S):
    if s not in _PROG:
        _PROG[s] = build_program(s)
    return _PROG[s]


def core_in_map(c, x, Wq, Wk, Wv, Wo, _shared={}):
    import ml_dtypes

    bf = ml_dtypes.bfloat16
    xid = id(x)
    if _shared.get("xid") != xid:
        _shared["xid"] = xid
        _shared["xb"] = np.ascontiguousarray(
            np.asarray(x, dtype=np.float32).astype(bf))
        _shared["m01"], _shared["mem"] = make_masks()
    h0 = c * HPC
    kv = (c * HPC) // (NQ // NKV)
    return {
        "xb": _shared["xb"],
        "wq": np.ascontiguousarray(
            np.asarray(Wq, np.float32)[:, h0 * DK:(h0 + HPC) * DK].astype(bf)),
        "wk": np.ascontiguousarray(
            np.asarray(Wk, np.float32)[:, kv * DK:(kv + 1) * DK].astype(bf)),
        "wv": np.ascontiguousarray(
            np.asarray(Wv, np.float32)[:, kv * DK:(kv + 1) * DK].astype(bf)),
        "wo": np.ascontiguousarray(
            np.asarray(Wo, np.float32)[h0 * DK:(h0 + HPC) * DK, :].astype(bf)),
        "mask01": _shared["m01"],
        "maskem": _shared["mem"],
    }


def kernel(x, Wq, Wk, Wv, Wo, **kw):
    from concourse.bass_utils import run_bass_kernel_spmd

    nc = _get_program(np.asarray(x).shape[1])
    in_maps = [core_in_map(c, x, Wq, Wk, Wv, Wo) for c in range(NCORES)]
    res = run_bass_kernel_spmd(nc, in_maps, core_ids=list(range(NCORES)), **kw)
    acc = np.zeros(np.asarray(x).shape, np.float64)
    for r in res.results:
        acc += r["out"]
    return acc.astype(np.float32)
